# revision 14
# baseline (speedup 1.0000x reference)
"""Trainium2 Bass kernel for nn_Conv2d_mvm (crossbar-quantized 3x3 conv).

The reference simulates a bit-sliced crossbar. Two key reductions:

1. The ADC clip [0, 511] can never bind (max per-xbar analog sum is
   128 rows * max slice digit 3 = 384), so the computation is exactly
   linear in the bit decompositions.

2. The weight reconstruction applies slice_w[0] = -2^14 to the whole
   MSB 2-bit digit, which is NOT true 2's complement: the conv uses
   effective weights  w_eff = wi - 32768*[wi < 0]  with
   wi = rne(4096*w) (|wi| <= ~1024 for this problem's weight scale).
   The input bit-streams reconstruct xi = rne(4096*x) exactly.

So:  acc = conv3x3(xi, wi) + conv3x3(xi, -32768*[wi < 0])
     out = clip(rne(acc / 4096), -32768, 32767) / 4096

Implementation (8 cores, data-parallel over batch x row-blocks):
  - core c handles batch c//4, output rows 8*(c%4) .. 8*(c%4)+8.
  - host pads x, packs the [64, 10*34] x-section and the [64, 9*64]
    (ci, kh, kw, co) weight block into one [64, 916] f32 input.
  - two partition-broadcast DMAs (0-stride dim duplicates the 64 DRAM
    rows onto both SBUF partition halves): x on the sync HWDGE ring,
    w on the scalar ring behind a junk activation that preloads the
    ACT table.
  - quantization in int16 (hardware f32->int16 converts are RNE and
    saturating, matching the reference's fixed-point rounding/clips):
      xh16 = rne(16*x)   (ACT)     xi16 = rne(4096*x)  (DVE)
      xbuf[0:64]   = fp16(256*xh16)            (ACT, exact)
      xbuf[64:128] = fp16(xi16 - 256*xh16)     (DVE stt, |.| <= 129)
      wi16 = rne(4096*w) (ACT)     wq = fp16(wi16)     (DVE, exact)
      wneg = -32768*[w < -1/8192]  (gpsimd, fp16 exact)
    so the 18 accumulating K=128 fp16 matmuls (9 taps x {mask, base})
    are exact: both matmul operand pairs multiply exactly-represented
    fp16 values into f32 PSUM.
  - epilogue: i16 = sat_int16(rne(psum/4096)) (one DVE op does the
    round AND the [-32768, 32767] clip), ot = f32(i16)/4096, one DMA.
  - PE warm-up: dummy matmuls on uninitialized SBUF start immediately
    so the PE p-state ramp (0.65 -> 1.2 -> 2.4 GHz after ~3us busy)
    completes before the real matmuls issue.
  - framework preamble register-moves (dead for this program) and the
    trailing all-engine barrier are stripped; the NEFF wrapper's own
    per-engine drain + rendezvous provides the end-of-program sync.

All arithmetic matching the reference happens on device; the host only
pads, shards, reshapes and gathers.
"""

from contextlib import ExitStack

import numpy as np

import concourse.bass as bass
import concourse.mybir as mybir
from concourse.bass_utils import run_bass_kernel_spmd

# fixed problem shape
B, C, H, W = 2, 64, 32, 32
COUT = 64
RPC = 8                    # output rows per core
SECR = RPC + 2             # padded rows per section
SECW = W + 2               # padded width
LEN = SECR * SECW          # 340
NOUT = (RPC - 1) * SECW + W  # 270 psum columns covering all valid pixels
OFFS = [dh * SECW + dw for dh in range(3) for dw in range(3)]
NW = 9 * COUT              # 576
NIN = LEN + NW             # 916 packed input columns

NDUM = 16                  # PE warm-up dummy matmuls (256 cols)
NTAIL = 10                 # fine-grained 64-col tail dummies
NDCOL = 256                # dummy matmul columns

F32 = mybir.dt.float32
F16 = mybir.dt.float16
I16 = mybir.dt.int16

_CACHED = None


def _build():
    nc = bass.Bass("TRN2", target_bir_lowering=False, debug=False, num_devices=8,
                   monotonic_sem_count=0)
    main = nc.m.functions[0].blocks[0]
    assert main.name == "main"
    n_preamble = len(main.instructions)

    xwin = nc.dram_tensor("xw", [2 * C, NIN], F32, kind="ExternalInput").ap()
    yout = nc.dram_tensor("y", [COUT, RPC, W], F32, kind="ExternalOutput").ap()

    with ExitStack() as ctx:
        xw2 = ctx.enter_context(nc.sbuf_tensor([2 * C, NIN], F32))
        xh16 = ctx.enter_context(nc.sbuf_tensor([2 * C, LEN], I16))
        xi16 = ctx.enter_context(nc.sbuf_tensor([2 * C, LEN], I16))
        wi16 = ctx.enter_context(nc.sbuf_tensor([2 * C, NW], I16))
        xbuf = ctx.enter_context(nc.sbuf_tensor([2 * C, LEN], F16))
        wball = ctx.enter_context(nc.sbuf_tensor([2 * C, 2 * NW], F16))
        i16o = ctx.enter_context(nc.sbuf_tensor([COUT, RPC * SECW], I16))
        ot = ctx.enter_context(nc.sbuf_tensor([COUT, RPC * W], F32))
        scr = ctx.enter_context(nc.sbuf_tensor([1, 8], F32))
        scr2 = ctx.enter_context(nc.sbuf_tensor([1, 8], F32))
        wdum = ctx.enter_context(nc.sbuf_tensor([2 * C, 2 * C], F16))
        mdum = ctx.enter_context(nc.sbuf_tensor([2 * C, NDCOL], F16))
        ps = ctx.enter_context(nc.psum_tensor([COUT, NOUT], F32))
        psd = ctx.enter_context(nc.psum_tensor([2 * C, NDCOL], F32))
        psd2 = ctx.enter_context(nc.psum_tensor([2 * C, 64], F32))
        s_a = ctx.enter_context(nc.semaphore())
        s_b = ctx.enter_context(nc.semaphore())
        s_act = ctx.enter_context(nc.semaphore())
        s_dve = ctx.enter_context(nc.semaphore())

        AL = mybir.AluOpType
        CP = mybir.ActivationFunctionType.Copy

        # ---- input DMAs: x section first (sync ring only; scalar ring
        # stays DMA-free so the ACT table + quant chain run unblocked) ----
        nc.sync.dma_start(xw2[:, 0:LEN], xwin[:, 0:LEN]).then_inc(s_a, 16)
        nc.sync.dma_start(xw2[:, LEN:NIN], xwin[:, LEN:NIN]).then_inc(s_b, 16)
        # ACT: junk activation triggers the ACT table load immediately
        nc.scalar.activation(scr[:], scr[:], CP, bias=0.0, scale=0.0)

        # ---- ACT: x high-part, xbuf top, w quant ----
        nc.scalar.wait_ge(s_a, 16)
        nc.scalar.activation(xh16[:], xw2[:, 0:LEN], CP, bias=0.0, scale=16.0).then_inc(s_act, 1)
        nc.scalar.activation(xbuf[0:C, :], xh16[0:C, :], CP, bias=0.0, scale=256.0).then_inc(s_act, 1)
        nc.scalar.wait_ge(s_b, 16)
        nc.scalar.activation(wi16[:], xw2[:, LEN:NIN], CP, bias=0.0, scale=4096.0).then_inc(s_act, 1)
        # wq = fp16(wi16), exact (|wi| <= ~1024)
        nc.scalar.activation(wball[:, 0:NW], wi16[:], CP, bias=0.0, scale=1.0).then_inc(s_act, 1)

        # ---- DVE: x low-part residual, mask weights, epilogue ----
        nc.vector.memset(scr2[:], 0.0)  # engine warm-up
        nc.vector.wait_ge(s_a, 16)
        nc.vector.tensor_scalar(xi16[C:2 * C, :], xw2[C:2 * C, 0:LEN], 4096.0, None, AL.mult)
        nc.vector.wait_ge(s_act, 1)
        # xbuf bottom = fp16(xi - 256*h), |.| <= 129 exact
        nc.vector.scalar_tensor_tensor(xbuf[C:2 * C, :], xh16[C:2 * C, :], -256.0,
                                       xi16[C:2 * C, :], AL.mult, AL.add).then_inc(s_dve, 1)
        # mask from raw w sign: wi = rne(4096*w) < 0  <=>  w < -1/8192
        nc.vector.wait_ge(s_b, 16)
        nc.vector.tensor_scalar(wball[:, NW:2 * NW], xw2[:, LEN:NIN],
                                -1.0 / 8192.0, -32768.0, AL.is_lt, AL.mult).then_inc(s_dve, 1)

        # ---- PE: warm-up on uninitialized SBUF, then the real conv.
        # Big dummies ramp the p-state; small tail dummies keep the PE
        # busy at fine granularity right up to the real matmuls (an idle
        # gap over ~1us drops the clock back to the mid p-state). ----
        for i in range(NDUM):
            nc.tensor.matmul(psd[:], wdum[:], mdum[:], start=(i == 0), stop=(i == NDUM - 1))
        for i in range(NTAIL):
            nc.tensor.matmul(psd2[:], wdum[:], mdum[:, 0:64],
                             start=(i == 0), stop=(i == NTAIL - 1))
        # mask group first: wneg (DVE, from raw w) is ready before wq
        nc.tensor.wait_ge(s_act, 2)
        nc.tensor.wait_ge(s_dve, 2)
        for d in range(9):
            nc.tensor.matmul(
                ps[:],
                wball[:, NW + d * COUT:NW + (d + 1) * COUT],
                xbuf[:, OFFS[d]:OFFS[d] + NOUT],
                start=(d == 0),
                stop=False,
            )
        nc.tensor.wait_ge(s_act, 4)
        for d in range(9):
            mm = nc.tensor.matmul(
                ps[:],
                wball[:, d * COUT:(d + 1) * COUT],
                xbuf[:, OFFS[d]:OFFS[d] + NOUT],
                start=False,
                stop=(d == 8),
            )
        mm.then_inc(s_act, 1)

        # ---- DVE epilogue: round+clip via saturating int16, rescale ----
        nc.vector.wait_ge(s_act, 5)
        nc.vector.tensor_scalar(i16o[:, 0:NOUT], ps[:], 1.0 / 4096.0, None, AL.mult)
        iv = i16o[:].rearrange("p (r c) -> p r c", c=SECW)[:, :, 0:W]
        ov = ot[:].rearrange("p (r c) -> p r c", c=W)
        nc.vector.tensor_scalar(ov, iv, 1.0 / 4096.0, None, AL.mult).then_inc(s_dve, 1)

        # ---- out DMA (NEFF wrapper's per-engine drain covers completion) ----
        nc.sync.wait_ge(s_dve, 3)
        nc.sync.dma_start(yout[:], ot[:].rearrange("p (r c) -> p r c", c=W)).then_inc(s_a, 16)

    # Strip framework const-AP memsets, the post-init barrier, and the
    # preamble register inits (all dead for this program; HW semaphores
    # are zero at NEFF load and re-zeroed by the NEFF epilogue).
    insts = main.instructions
    pre = [
        ins for ins in insts[:n_preamble]
        if type(ins).__name__ not in (
            "InstMemset", "InstDrain", "InstEventSemaphore", "InstRegisterMove")
    ]
    main.instructions = pre + insts[n_preamble:]

    return nc


def _get_nc():
    global _CACHED
    if _CACHED is None:
        _CACHED = _build()
    return _CACHED


def _shard_inputs(x, weight):
    xpad = np.pad(np.ascontiguousarray(x, dtype=np.float32),
                  ((0, 0), (0, 0), (1, 1), (1, 1)))
    wre = np.asarray(weight, dtype=np.float32).transpose(1, 2, 3, 0).reshape(C, NW)
    in_maps = []
    for c in range(8):
        b, q = divmod(c, 4)
        sec = xpad[b, :, RPC * q:RPC * q + SECR, :].reshape(C, LEN)
        xw = np.concatenate([sec, wre], axis=1)
        in_maps.append({"xw": np.ascontiguousarray(np.concatenate([xw, xw], axis=0))})
    return in_maps


def kernel(x, weight):
    nc = _get_nc()
    in_maps = _shard_inputs(x, weight)
    res = run_bass_kernel_spmd(nc, in_maps, core_ids=list(range(8)))
    out = np.empty((B, COUT, H, W), dtype=np.float32)
    for c in range(8):
        b, q = divmod(c, 4)
        out[b, :, RPC * q:RPC * q + RPC, :] = res.results[c]["y"]
    return out
